# revision 33
# baseline (speedup 1.0000x reference)
"""Trainium2 Bass kernel for nn_Bottleneck_57561151701110 (SAM pairwise
bottleneck block). Data-parallel over batch: 8 images -> 8 NeuronCores.

Per-core pipeline (one 256x56x56 image):
  conv1/2/3 (1x1, PE f32r) -> pairwise feat = x1' - shift(x2') (DVE) ->
  w1/w2 small matmuls in 4-row-band blockdiag layout (PE) -> softmax over
  k=9 (ACT exp, PE sum, DVE recip) -> grouped aggregation as 9 broadcast
  multiplies (DVE bf16 2x) + PE identity-accumulate -> leaky -> conv_out
  (PE) + residual -> out.

BN layers are folded into conv weights/biases on the host. The position
branch (tiny, batch-independent) is precomputed on the host and enters
the w1 matmul as an extra low-rank accumulate.
"""

import os
import sys

for _p in ("/opt/trn_rl_repo", os.path.expanduser("~/.axon_site/_ro/trn_rl_repo")):
    if os.path.isdir(_p) and _p not in sys.path:
        sys.path.insert(0, _p)

from contextlib import ExitStack

import numpy as np
import ml_dtypes

import concourse.bass as bass
import concourse.bacc as bacc
import concourse.tile as tile
from concourse import mybir
from concourse.bass_utils import run_bass_kernel_spmd

dt = mybir.dt
ALU = mybir.AluOpType
ACTF = mybir.ActivationFunctionType

B, CIN, H, W = 8, 256, 56, 56
NPIX = H * W            # 3136
REL, MID, OUT = 32, 256, 256
SHARE = 8
NG = MID // SHARE       # 32 groups
NB = 4                  # row bands
BH = H // NB            # 14 rows per band
Q = BH * W              # 784 band pixels
BR = BH + 2             # 16 padded band rows
NEG = 0.01
BN_EPS = 1e-5
CCH = 448               # conv free chunk (8 rows)
NCH = NPIX // CCH       # 7
OFFS = [(dh, dw) for dh in (-1, 0, 1) for dw in (-1, 0, 1)]
HQ = Q // 2  # k = 3(dh+1)+(dw+1)

F32, F32R, BF16 = dt.float32, dt.float32r, dt.bfloat16

_CACHE = {}


# ----------------------------------------------------------------- host prep
def _position(h, w):
    loc_w = np.tile(np.linspace(-1.0, 1.0, w, dtype=np.float32)[None, :], (h, 1))
    loc_h = np.tile(np.linspace(-1.0, 1.0, h, dtype=np.float32)[:, None], (1, w))
    return np.stack([loc_w, loc_h], axis=0)  # (2, H, W)


def _host_consts(inp):
    f32 = np.float32
    inv_a = (inp["bna_g"] / np.sqrt(inp["bna_v"] + BN_EPS)).astype(f32)
    beta_a = (inp["bna_b"] - inp["bna_m"] * inv_a).astype(f32)
    inv_b = (inp["bnb_g"] / np.sqrt(inp["bnb_v"] + BN_EPS)).astype(f32)
    beta_b = (inp["bnb_b"] - inp["bnb_m"] * inv_b).astype(f32)

    w1c = inp["conv1_w"] * inv_a[:REL, None]
    b1 = inp["conv1_b"] * inv_a[:REL] + beta_a[:REL]
    w2c = inp["conv2_w"] * inv_a[:REL, None]
    b2 = inp["conv2_b"] * inv_a[:REL]

    # conv1+conv2 fused stationary: (256, 64), chunked over K
    c12 = np.concatenate([w1c, w2c], axis=0).T.astype(f32).copy()  # (256,64)
    bias12 = np.zeros((128, 1), f32)
    bias12[:REL, 0] = b1
    bias12[REL:2 * REL, 0] = b2

    c3 = inp["conv3_w"].T.astype(f32).copy()    # (256,256) lhsT
    co = inp["convo_w"].T.astype(f32).copy()    # (256,256) lhsT

    # W1' with bnb scale folded
    w1p = (inp["w1"] * inv_b[:, None]).astype(f32)  # (32, 34)
    w1a, w1b = w1p[:, :REL], w1p[:, REL:]
    lhsT_w1 = np.zeros((128, 128), f32)
    lhsT_pos = np.zeros((8, 128), f32)
    lhsT_w2 = np.zeros((128, 128), f32)
    for b in range(NB):
        lhsT_w1[32 * b:32 * b + 32, 32 * b:32 * b + 32] = w1a.T  # [c, o]
        lhsT_pos[2 * b:2 * b + 2, 32 * b:32 * b + 32] = w1b.T    # [c2, o]
        lhsT_w2[32 * b:32 * b + 32, 32 * b:32 * b + 32] = inp["w2"].T  # [o, g]

    betab = np.tile(beta_b, NB).astype(f32).reshape(128, 1)
    w2bv = np.tile(inp["w2_b"], NB).astype(f32).reshape(128, 1)

    # position branch, batch independent: posr[2b+c2, 784k+q] =
    # relu(inv_a[32+c2]*subp[c2,k,band b pix q] + beta_a[32+c2])
    pos = _position(H, W)
    pc = np.einsum("oc,chw->ohw", inp["convp_w"], pos) + inp["convp_b"][:, None, None]
    pcp = np.pad(pc, ((0, 0), (1, 1), (1, 1)))
    posr = np.zeros((8, 9 * Q), f32)
    for k, (dh, dw) in enumerate(OFFS):
        sub = pc - pcp[:, 1 + dh:1 + dh + H, 1 + dw:1 + dw + W]  # (2,56,56)
        v = np.maximum(inv_a[REL:, None, None] * sub + beta_a[REL:, None, None], 0.0)
        vb = v.reshape(2, NB, BH, W)  # (c2, b, r, w)
        for b in range(NB):
            posr[2 * b:2 * b + 2, Q * k:Q * (k + 1)] = vb[:, b].reshape(2, Q)

    vecs = np.zeros((128, 8), f32)
    vecs[:, 0:1] = bias12
    vecs[:128, 1] = inp["conv3_b"][:128]
    vecs[:128, 2] = inp["conv3_b"][128:]
    vecs[:, 3:4] = betab
    vecs[:, 4:5] = w2bv

    biaso_l = inp["convo_b"].astype(f32).reshape(1, 256).copy()
    bf16 = ml_dtypes.bfloat16

    return {
        "c12": c12, "c3": c3,
        "co": co.astype(bf16),
        "lhsT_w1": lhsT_w1.astype(bf16), "lhsT_pos": lhsT_pos.astype(bf16),
        "lhsT_w2": lhsT_w2.astype(bf16),
        "posr": posr.astype(bf16), "vecs": vecs, "biaso_l": biaso_l,
        "identb": np.eye(128, dtype=bf16),
        "sel": _band_selector().astype(bf16),
        "identf": np.eye(128, dtype=f32),
        "ones": np.ones((1, CCH), f32),
    }


def _band_selector():
    # sel[c or 32+c, 128*b + (32*b + c)] = 1 : lhsT that scatters a
    # 32-channel tile into partition block b of a 128-partition band tile.
    # Rows 32..63 duplicate rows 0..31 so the x2 remap (rhs partitions
    # 32..63) has a matching lhsT base partition.
    s = np.zeros((64, 4 * 128), np.float32)
    for b in range(NB):
        for c in range(32):
            s[c, 128 * b + 32 * b + c] = 1.0
            s[32 + c, 128 * b + 32 * b + c] = 1.0
    return s


# ------------------------------------------------------------ program build
def _build_program():
    nc = bacc.Bacc("TRN2", target_bir_lowering=False, debug=False,
                   enable_asserts=False, num_devices=8)

    din = {}
    def dram_in(name, shape, dtype):
        din[name] = nc.dram_tensor(name, list(shape), dtype, kind="ExternalInput").ap()
        return din[name]

    xin = dram_in("xin", (CIN, NPIX), F32R)
    c12d = dram_in("c12", (CIN, 64), F32R)
    c3d = dram_in("c3", (CIN, CIN), F32R)
    cod = dram_in("co", (CIN, CIN), BF16)
    w1d = dram_in("lhsT_w1", (128, 128), BF16)
    posd = dram_in("lhsT_pos", (8, 128), BF16)
    w2d = dram_in("lhsT_w2", (128, 128), BF16)
    posrd = dram_in("posr", (8, 9 * Q), BF16)
    vecsd = dram_in("vecs", (128, 8), F32)
    biasod = dram_in("biaso_l", (1, 256), F32R)
    identd = dram_in("identb", (128, 128), BF16)
    onesd = dram_in("ones", (1, CCH), F32R)
    identfd = dram_in("identf", (128, 128), F32R)
    seld = dram_in("sel", (64, 4 * 128), BF16)

    outd = nc.dram_tensor("out", [CIN, NPIX], F32, kind="ExternalOutput").ap()

    # DRAM scratch
    # per-band x3 scratch: channel-plane = 16 padded rows x 58 cols (halo dup)
    x3b = [nc.dram_tensor(f"x3b{b}", [CIN, BR * 58], BF16).ap() for b in range(NB)]
    samd = nc.dram_tensor("samd", [CIN, NPIX], BF16).ap()

    _eng = [nc.sync, nc.gpsimd, nc.scalar]
    _ei = [0]
    def dma(dst, src):
        # spread bulk transfers across all DMA-capable engines' queues
        _ei[0] += 1
        _eng[_ei[0] % 3].dma_start(dst, src)

    with tile.TileContext(nc) as tc, ExitStack() as ctx:
        nc_ = tc.nc

        cpool = ctx.enter_context(tc.tile_pool(name="consts", bufs=1))
        sb1 = ctx.enter_context(tc.tile_pool(name="sb1", bufs=2))
        sbf = ctx.enter_context(tc.tile_pool(name="sbf", bufs=1))
        sbh = ctx.enter_context(tc.tile_pool(name="sbh", bufs=2))
        epool = ctx.enter_context(tc.tile_pool(name="epool", bufs=9))
        xgpool = ctx.enter_context(tc.tile_pool(name="xgpool", bufs=3))
        fpool = ctx.enter_context(tc.tile_pool(name="fpool", bufs=2))


        # ---- const loads (dim0 = partitions; K-chunks live on the free axis)
        c12t = cpool.tile([128, 2, 64], F32R, tag="c12")
        c3t = cpool.tile([128, 2, CIN], F32R, tag="c3")
        cot = cpool.tile([128, 2, CIN], BF16, tag="co")
        for kc in range(2):
            dma(c12t[:, kc, :], c12d[128 * kc:128 * (kc + 1), :])
            dma(c3t[:, kc, :], c3d[128 * kc:128 * (kc + 1), :])
            dma(cot[:, kc, :], cod[128 * kc:128 * (kc + 1), :])
        w1t = cpool.tile([128, 128], BF16, tag="w1t")
        dma(w1t[:], w1d[:])
        post = cpool.tile([8, 128], BF16, tag="post")
        dma(post[:], posd[:])
        w2t = cpool.tile([128, 128], BF16, tag="w2t")
        dma(w2t[:], w2d[:])
        vecst = cpool.tile([128, 8], F32, tag="vecst")
        dma(vecst[:], vecsd[:])
        biasot = cpool.tile([1, 256], F32R, tag="biasot")
        dma(biasot[:], biasod[:])
        identt = cpool.tile([128, 128], BF16, tag="identt")
        dma(identt[:], identd[:])
        identft = cpool.tile([128, 128], F32R, tag="identft")
        dma(identft[:], identfd[:])
        selt = cpool.tile([64, 4 * 128], BF16, tag="selt")
        dma(selt[:], seld[:])
        onest = cpool.tile([1, CCH], F32R, tag="onest")
        dma(onest[:], onesd[:])

        # position-branch tiles: pure inputs, load first on the scalar queues
        prpool = ctx.enter_context(tc.tile_pool(name="prpool", bufs=9))
        prts = []
        for k in range(9):
            prt = prpool.tile([8, Q], BF16, tag="prt")
            nc.scalar.dma_start(prt[:], posrd[:, Q * k:Q * (k + 1)])
            prts.append(prt)

        # ---- input x, loaded per conv chunk for load/compute overlap
        xscope = ExitStack()
        xpool = xscope.enter_context(tc.tile_pool(name="xin", bufs=8))
        xt = {}
        for c in range(NCH):
            for t in range(2):
                xx = xpool.tile([128, CCH], F32R, tag="xt")
                dma(xx[:], xin[128 * t:128 * (t + 1), CCH * c:CCH * (c + 1)])
                xt[(t, c)] = xx


        # ---- phase A: conv1+conv2 (all chunks), then band remap, then conv3.
        # Order matters: the PE stream runs conv12 -> remap -> conv3 so the
        # pairwise/logits phase can start while conv3 is still running.
        x12s = sbf.tile([64, NPIX], BF16, tag="x12s")
        pscope1 = ExitStack()
        pp1 = pscope1.enter_context(tc.tile_pool(name="pp1", bufs=3, space="PSUM"))
        zt = cpool.tile([128, 64], BF16, tag="zt")
        nc_.gpsimd.memset(zt[:], 0.0)
        for t in range(2):
            tsl = slice(128 * t, 128 * (t + 1))
            dma(x3b[0][tsl, 0:58], zt[:, 0:58])      # band0 top halo row = 0
            dma(x3b[NB - 1][tsl, 15 * 58:16 * 58], zt[:, 0:58])  # band3 bottom
        for c in range(NCH):
            sl = slice(CCH * c, CCH * (c + 1))
            ps = pp1.tile([64, CCH], F32, tag="ps12")
            nc_.tensor.matmul(ps[:], c12t[:, 0, :], xt[(0, c)][:],
                              start=True, stop=False)
            nc_.tensor.matmul(ps[:], c12t[:, 1, :], xt[(1, c)][:],
                              start=False, stop=True)
            nc_.vector.tensor_scalar(x12s[:, sl], ps[:], vecst[0:64, 0:1],
                                     None, op0=ALU.add)

        # ---- band tiles for feat path, via selector matmuls (partition remap)
        x1b = sbf.tile([128, BH, W], BF16, tag="x1b")
        x2b = sbf.tile([128, BR, W + 2], BF16, tag="x2b")
        nc_.gpsimd.memset(x2b[:], 0.0)
        with tc.tile_pool(name="ppr", bufs=1, space="PSUM") as ppr:
            for b in range(NB):
                lhs = selt[0:32, 128 * b:128 * (b + 1)]
                lhs2 = selt[32:64, 128 * b:128 * (b + 1)]
                psb = ppr.tile([128, 1024], F32, tag="psb")
                for j, (o0, n) in enumerate(((0, HQ), (HQ, HQ))):
                    nc_.tensor.matmul(psb[:, 512 * j:512 * j + n], lhs,
                                      x12s[0:REL, Q * b + o0:Q * b + o0 + n],
                                      start=True, stop=True)
                    nc_.vector.tensor_copy(
                        x1b[:, 7 * j:7 * j + 7, :],
                        psb[:, 512 * j:512 * j + n]
                        .rearrange("p (r w) -> p r w", w=W))
                rlo = 1 if b == 0 else 0
                rhi = 15 if b == NB - 1 else 16
                p0 = (BH * b - 1 + rlo) * W
                n1 = 448
                n2 = (rhi - rlo) * W - n1
                psb2 = ppr.tile([128, 1024], F32, tag="psb")
                for j, (o0, n) in enumerate(((0, n1), (n1, n2))):
                    nc_.tensor.matmul(psb2[:, 512 * j:512 * j + n], lhs2,
                                      x12s[REL:2 * REL, p0 + o0:p0 + o0 + n],
                                      start=True, stop=True)
                    r0 = rlo + (o0 // W)
                    nc_.vector.tensor_copy(
                        x2b[:, r0:r0 + n // W, 1:1 + W],
                        psb2[:, 512 * j:512 * j + n]
                        .rearrange("p (r w) -> p r w", w=W))

        # ---- phase B: conv3 -> per-band x3 scratch (bf16)
        for c in range(NCH):
            for t in range(2):
                ps3 = pp1.tile([128, CCH], F32, tag="ps3")
                nc_.tensor.matmul(ps3[:], c3t[:, 0, 128 * t:128 * (t + 1)],
                                  xt[(0, c)][:], start=True, stop=False)
                nc_.tensor.matmul(ps3[:], c3t[:, 1, 128 * t:128 * (t + 1)],
                                  xt[(1, c)][:], start=False, stop=True)
                # 58-wide rows with zero border cols -> one contiguous store
                x3s = sb1.tile([128, 8, 58], BF16, tag="x3s")
                nc_.gpsimd.memset(x3s[:, :, 0:1], 0.0)
                nc_.gpsimd.memset(x3s[:, :, 57:58], 0.0)
                nc_.scalar.activation(x3s[:, :, 1:57],
                                      ps3[:].rearrange("p (r w) -> p r w", w=W),
                                      ACTF.Identity, bias=vecst[:, 1 + t:2 + t])
                for b in range(NB):
                    lo = max(8 * c, BH * b - 1)
                    hi = min(8 * (c + 1), BH * b + BH + 1)
                    if lo >= hi:
                        continue
                    dma(x3b[b][128 * t:128 * (t + 1),
                               (lo - (BH * b - 1)) * 58:(hi - (BH * b - 1)) * 58],
                        x3s[:, lo - 8 * c:hi - 8 * c, :])
        pscope1.close()
        xscope.close()

        # ---- x3 group-band tiles: A = unshifted (serves dw=+-1),
        # B = shifted one col right (serves dw=0) so every TT window is
        # 4-byte aligned (2x DVE mode). Layout (128=(b,g), s, 16, 58).
        xgA = xgpool.tile([128, SHARE, BR, 58], BF16, tag="xgA")
        xgB = xgpool.tile([128, SHARE, BR, 58], BF16, tag="xgB")
        nc_.gpsimd.memset(xgB[:], 0.0)
        for b in range(NB):
            psl = slice(32 * b, 32 * b + 32)
            srcv = x3b[b][:].rearrange("(g s) f -> g (s f)", s=SHARE)
            dma(xgA[psl].rearrange("p s r w -> p (s r w)"), srcv)
            dma(xgB[psl].rearrange("p s r w -> p (s r w)")[:, 1:],
                srcv[:, 0:SHARE * BR * 58 - 1])
        xg = {-1: xgA, 0: xgB, 1: xgA}
        xgo = {-1: 0, 0: 2, 1: 2}  # col offset of the dh-window per dw

        # ---- phase C: per-k logits + exp + Z
        pscope2 = ExitStack()
        ppz = pscope2.enter_context(tc.tile_pool(name="ppz", bufs=1, space="PSUM"))
        pscope3 = ExitStack()
        pph = pscope3.enter_context(tc.tile_pool(name="pph", bufs=3, space="PSUM"))
        ek = []
        zps = ppz.tile([128, 1024], F32, tag="zps")
        wsl = [slice(0, 512), slice(512, Q)]
        for k, (dh, dw) in enumerate(OFFS):
            fs = fpool.tile([128, BH, W], F32, tag="fs")
            nc_.vector.tensor_tensor(
                fs[:], x1b[:],
                x2b[:, 1 + dh:1 + dh + BH, 1 + dw:1 + dw + W],
                ALU.subtract)
            fr = fpool.tile([128, Q], BF16, tag="fr")
            nc_.vector.tensor_scalar(fr[:].rearrange("p (r w) -> p r w", w=W),
                                     fs[:], 0.0, None, op0=ALU.max)
            prt = prts[k]
            hps = pph.tile([128, 1024], F32, tag="hw")
            for s in wsl:
                nc_.tensor.matmul(hps[:, s], w1t[:], fr[:, s],
                                  start=True, stop=False)
                nc_.tensor.matmul(hps[:, s], post[:], prt[:, s],
                                  start=False, stop=True)
            hp = sbh.tile([128, Q], BF16, tag="hp")
            nc_.scalar.activation(hp[:], hps[:, 0:Q], ACTF.Relu,
                                  bias=vecst[:, 3:4])
            wps = pph.tile([128, 1024], F32, tag="hw")
            for s in wsl:
                nc_.tensor.matmul(wps[:, s], w2t[:], hp[:, s],
                                  start=True, stop=True)
            e = epool.tile([128, Q], BF16, tag="e")
            nc_.scalar.activation(e[:], wps[:, 0:Q], ACTF.Exp,
                                  bias=vecst[:, 4:5])
            for s in wsl:
                nc_.tensor.matmul(zps[:, s], identt[:], e[:, s],
                                  start=(k == 0), stop=(k == 8))
            ek.append(e)
        pscope3.close()

        # ---- phase D: 1/Z (leaky is positively homogeneous, so the
        # softmax divide is applied after leaky, as a broadcast multiply)
        rzpool = ctx.enter_context(tc.tile_pool(name="rzpool", bufs=1))
        rz = rzpool.tile([128, Q], F32, tag="rz")
        nc_.vector.reciprocal(rz[:], zps[:, 0:Q])
        rz16 = rzpool.tile([128, Q], BF16, tag="rz16")
        nc_.vector.tensor_copy(rz16[:], rz[:])
        pscope2.close()

        # ---- phase E: aggregation in quarter rounds (s-quad x half-band)
        # so conv_out can start on finished half-bands while agg continues.
        NSQ = SHARE // 2  # 4 s per quad
        HBW = Q // 2      # 392 half-band pixels (7 rows)
        pkpool = ctx.enter_context(tc.tile_pool(name="pkpool", bufs=2))
        sqpool = ctx.enter_context(tc.tile_pool(name="sqpool", bufs=2))
        ppo = ctx.enter_context(tc.tile_pool(name="ppo", bufs=3, space="PSUM"))
        rxpool = ctx.enter_context(tc.tile_pool(name="rxpool", bufs=4))
        smcpool = ctx.enter_context(tc.tile_pool(name="smcpool", bufs=4))
        obpool = ctx.enter_context(tc.tile_pool(name="obpool", bufs=4))
        pscope4 = ExitStack()
        pps = pscope4.enter_context(tc.tile_pool(name="pps", bufs=1, space="PSUM"))
        samdv = samd[:].rearrange("(g s) (b h q2) -> g s b h q2",
                                  s=SHARE, b=NB, h=2)

        def convo_halfband(qp, b):
            # conv_out on half-band (qp, b): 392 pixels, all 256 channels
            po = 784 * b + HBW * qp
            smc = []
            for t in range(2):
                s_ = smcpool.tile([128, HBW], BF16, tag="smc")
                dma(s_[:], samd[128 * t:128 * (t + 1), po:po + HBW])
                smc.append(s_)
            for t in range(2):
                ps = ppo.tile([128, HBW], F32, tag="pso")
                nc_.tensor.matmul(ps[:], cot[:, 0, 128 * t:128 * (t + 1)],
                                  smc[0][:], start=True, stop=False)
                nc_.tensor.matmul(ps[:], cot[:, 1, 128 * t:128 * (t + 1)],
                                  smc[1][:], start=False, stop=False)
                nc_.tensor.matmul(ps[:], biasot[0:1, 128 * t:128 * (t + 1)],
                                  onest[:, 0:HBW], start=False, stop=True)
                r = sb1.tile([128, HBW], F32, tag="rr")
                nc_.scalar.activation(r[:], ps[:], ACTF.Relu,
                                      scale=-(1.0 - NEG))
                rxt = rxpool.tile([128, HBW], F32R, tag="rxt")
                dma(rxt[:], xin[128 * t:128 * (t + 1), po:po + HBW])
                nc_.tensor.matmul(ps[:], identft[:], rxt[:],
                                  start=False, stop=True,
                                  skip_group_check=True)
                o = obpool.tile([128, HBW], F32, tag="oo")
                nc_.vector.tensor_tensor(o[:], ps[:], r[:], ALU.add)
                dma(outd[128 * t:128 * (t + 1), po:po + HBW], o[:])

        for qp in range(2):           # half-band (row halves 0-6 / 7-13)
            for sq in range(2):       # s-quad
                sam = pps.tile([128, 2048], F32, tag="sam")
                samv = sam[:].rearrange("p (a j) -> p a j", j=512)[:, :, 0:HBW]
                for k, (dh, dw) in enumerate(OFFS):
                    pk = pkpool.tile([128, NSQ, 7, W], BF16, tag="pk")
                    co_ = xgo[dw]
                    r0 = 1 + dh + 7 * qp
                    nc_.vector.tensor_tensor(
                        pk[:],
                        xg[dw][:, NSQ * sq:NSQ * (sq + 1), r0:r0 + 7,
                               co_:co_ + W],
                        ek[k][:].rearrange("p (r w) -> p r w", w=W)
                        [:, 7 * qp:7 * qp + 7, :].unsqueeze(1)
                        .broadcast_to((128, NSQ, 7, W)),
                        ALU.mult)
                    pkf = pk[:].rearrange("p a r w -> p (a r w)")
                    for c in range(4):
                        nc_.tensor.matmul(
                            sam[:, 512 * c:512 * c + HBW], identt[:],
                            pkf[:, HBW * c:HBW * (c + 1)],
                            start=(k == 0), stop=(k == 8))
                rq = sqpool.tile([128, NSQ, HBW], BF16, tag="rq")
                nc_.scalar.activation(rq[:], samv, ACTF.Relu,
                                      scale=-(1.0 - NEG))
                sqr = sqpool.tile([128, NSQ, HBW], BF16, tag="sqr")
                nc_.vector.tensor_tensor(sqr[:], samv, rq[:], ALU.add)
                sq_ = sqpool.tile([128, NSQ, HBW], BF16, tag="sq")
                nc_.vector.tensor_tensor(
                    sq_[:], sqr[:],
                    rz16[:, HBW * qp:HBW * (qp + 1)].unsqueeze(1)
                    .broadcast_to((128, NSQ, HBW)),
                    ALU.mult)
                for b in range(NB):
                    dma(samdv[:, NSQ * sq:NSQ * (sq + 1), b, qp, :],
                        sq_[32 * b:32 * b + 32])
            for b in range(NB):
                convo_halfband(qp, b)
        pscope4.close()

    nc.compile()
    return nc


# --------------------------------------------------------------- entrypoint
def _get_program():
    if "nc" not in _CACHE:
        _CACHE["nc"] = _build_program()
    return _CACHE["nc"]


def kernel(**inputs):
    inputs = {k: np.asarray(v) for k, v in inputs.items()}
    consts = _host_consts(inputs)
    nc = _get_program()
    x = inputs["x"].reshape(B, CIN, NPIX).astype(np.float32)
    in_maps = []
    for b in range(B):
        m = {k: v for k, v in consts.items()}
        m["xin"] = x[b]
        in_maps.append(m)
    res = run_bass_kernel_spmd(nc, in_maps, list(range(B)))
    out = np.stack([res.results[i]["out"] for i in range(B)])
    return out.reshape(B, CIN, H, W).astype(np.float32)


def kernel_traced(**inputs):
    """Like kernel() but with NTFF tracing; returns (out, BassKernelResults)."""
    inputs = {k: np.asarray(v) for k, v in inputs.items()}
    consts = _host_consts(inputs)
    nc = _get_program()
    x = inputs["x"].reshape(B, CIN, NPIX).astype(np.float32)
    in_maps = []
    for b in range(B):
        m = {k: v for k, v in consts.items()}
        m["xin"] = x[b]
        in_maps.append(m)
    res = run_bass_kernel_spmd(nc, in_maps, list(range(B)), trace=True)
    out = np.stack([res.results[i]["out"] for i in range(B)])
    return out.reshape(B, CIN, H, W).astype(np.float32), res
